# revision 19
# baseline (speedup 1.0000x reference)
"""Multi-head attention forward on 8 Trainium2 NeuronCores — v2.

Problem (hardcoded): B=2, S=2048, D=1024, H=16, HD=64
    q = relu(x @ Wq + bq); k = relu(x @ Wk + bk); v = relu(x @ Wv + bv)
    attn = softmax(q k^T / sqrt(HD)) per (batch, head)
    out = relu((attn @ v) @ Wo + bo)

Sharding: head-parallel attention (2 heads/core, both batches); AllToAlls
re-shard ctx^T to token-parallel output projection (128-token blocks per
core per half-batch).  Host packs inputs into DMA-friendly layouts and
reassembles the output.

Key differences vs v1:
  - q/k stored stacked [128, T] so the two heads' score matmuls (K=64) run
    CONCURRENTLY on disjoint PE row groups (auto tile_position via
    base_partition 0/64).
  - ctx matmul in fp8e4 with perf_mode=DoubleRow: exp writes pt [128,2,512]
    fp8 (key-block pair interleave), V_aug stored fp8 [128,2,128] -> one
    DR matmul covers 2 key blocks per head at 0.5 cyc/row.
  - softmax normalize straight out of PSUM via reciprocal_approx_fast
    (~5x faster than the iterative divide).
  - emission order: batch-0 projections lead in; batch-1 projections ride
    as fine-grained fillers inside attention(0) (generator units of ~4
    matmuls so a lagging PE is never blocked long); attention(1) is kept
    light so its exp pace is not PE-gated; all output projections run in
    the tail where batch-0's blocks cover the last AllToAll's latency.
    NOTE: same-batch k/v may NOT ride as fillers inside their own
    attention phase - that raced (nondeterministic wrong results).
  - per-half-batch AllToAlls (4 x 256KB) with token ownership re-mapped so
    every core receives one 128-token block per collective; the last
    collective only carries the final quarter of the tokens.
  - x loaded via 8 single-DMA chunks with 8KB/partition contiguous lines
    (host pre-packs), so the first projection starts right after launch.
"""

import os
import sys

import numpy as np

for _p in ("/opt/trn_rl_repo",):
    if os.path.isdir(_p) and _p not in sys.path:
        sys.path.append(_p)

import ml_dtypes

B, S, D, H = 2, 2048, 1024, 16
HD = D // H          # 64
NCORES = 8
T = B * S            # 4096
DC = D // NCORES     # 128 cols per core (2 heads)
P = 128
KT = D // P          # 8 contraction tiles
SBQ = S // 512       # 4 query chunks per batch
KB = S // P          # 16 key blocks per batch
KBP = KB // 2        # 8 key-block pairs per batch
HALF = 2             # halves per batch (qc pairs) for the A2A
TOK = 128            # tokens per (core, batch, half)

_bf = ml_dtypes.bfloat16
_f8 = ml_dtypes.float8_e4m3

PROFILE = False
PROFILE_CORES = [0]
LAST_RESULTS = None

CTX_FP8 = True      # fp8e4 DoubleRow ctx matmul (else bf16, per-kb)
PACK_SCORES = True  # heads stacked [128,T]; score MMs on row groups 0/64
SCORES_FP8 = False  # fp8e4 DoubleRow score matmuls (measured: correct at
                    # rel 0.0030 but SLOWER, 338us vs 290 — the per-MM
                    # 256-col DR LDWEIGHTS dominates the shortened matmuls)
FAST_RECIP = True
PAD_PE = False      # pad PE slots with dummy matmuls (measured: hurts)
DEBUG_DUMP = False  # add a dbg output with the received ctxt tile

_CACHE = {}


def _build(wb_qk, wb_v, wb_o):
    import concourse.mybir as mybir
    import concourse.tile as tile
    from concourse import bacc
    from concourse.bass import ds, ts
    from contextlib import ExitStack

    f32 = mybir.dt.float32
    bf16 = mybir.dt.bfloat16
    fp8 = mybir.dt.float8e4
    DT = bf16
    AF = mybir.ActivationFunctionType
    DR = mybir.MatmulPerfMode.DoubleRow

    nc = bacc.Bacc("TRN2", target_bir_lowering=False, debug=False,
                   num_devices=NCORES)

    # host-packed inputs
    xq = nc.dram_tensor("xq", [SBQ * B, P, KT * 512], DT, kind="ExternalInput")
    wq = nc.dram_tensor("wq", [P, KT, DC], DT, kind="ExternalInput")
    wk = nc.dram_tensor("wk", [P, KT, DC], DT, kind="ExternalInput")
    wv = nc.dram_tensor("wv", [P, KT, DC], DT, kind="ExternalInput")
    wo = nc.dram_tensor("wo", [P, KT, D], DT, kind="ExternalInput")
    bqd = nc.dram_tensor("bqv", [1, DC], DT, kind="ExternalInput")
    bkd = nc.dram_tensor("bkv", [1, DC], DT, kind="ExternalInput")
    bvd = nc.dram_tensor("bvv", [1, DC], DT, kind="ExternalInput")
    bod = nc.dram_tensor("bov", [1, D], DT, kind="ExternalInput")
    out = nc.dram_tensor("out", [B * HALF * TOK, D], f32, kind="ExternalOutput")
    dbg = nc.dram_tensor("dbg", [P, 8192], DT,
                         kind="ExternalOutput") if DEBUG_DUMP else None

    with tile.TileContext(nc) as tc, ExitStack() as ctx:
        sb = ctx.enter_context(tc.tile_pool(name="persist", bufs=1))
        dram = ctx.enter_context(tc.tile_pool(name="dram", bufs=1, space="DRAM"))
        psum = ctx.enter_context(tc.tile_pool(name="psum", bufs=1, space="PSUM"))
        ptp = ctx.enter_context(tc.tile_pool(name="ptp", bufs=3))
        nrm = ctx.enter_context(tc.tile_pool(name="nrm", bufs=4))
        osb_p = ctx.enter_context(tc.tile_pool(name="osbp", bufs=3))

        PDT = fp8 if CTX_FP8 else DT
        xts = sb.tile([P, SBQ * B, KT, 512], DT)
        if SCORES_FP8:
            # [p = h*32 + p', jj, t]; value = proj[token t, col h*64+jj*32+p']
            qt = sb.tile([2 * 32, 2, T], fp8)
            kt = sb.tile([2 * 32, 2, T], fp8)
        else:
            qt = sb.tile([P, T], DT)
            kt = sb.tile([P, T], DT)
        if not PACK_SCORES:
            qth = [sb.tile([HD, T], DT, name=f"qth{h}") for h in range(2)]
            kth = [sb.tile([HD, T], DT, name=f"kth{h}") for h in range(2)]
        # V_aug: [key, pair, head, j, col]; col 0:64 = ones (so the softmax
        # denominator lands at PSUM partitions 0:63 where the base-0-only
        # reciprocal_approx_fast can read it), col 64:128 = V
        va = sb.tile([P, B * KBP, 2, 2, P], PDT)
        wq_s = sb.tile([P, KT, DC], DT)
        wk_s = sb.tile([P, KT, DC], DT)
        wv_s = sb.tile([P, KT, DC], DT)
        wo_s = sb.tile([P, KT, D], DT)
        # received ctx^T: [row-of-src, b*HALF+half, src(kti), tok]
        ctxt = sb.tile([P, B * HALF, KT, TOK], DT)
        ones_r = sb.tile([1, 512], DT)
        bq_s = sb.tile([1, DC], DT)
        bk_s = sb.tile([1, DC], DT)
        bv_s = sb.tile([1, DC], DT)
        bo_s = sb.tile([1, D], DT)
        warm = sb.tile([1, 32], f32)
        ebias = sb.tile([P, 1], f32)

        nc.vector.memset(ebias[:], -2.0)
        nc.vector.memset(ones_r[:], 1.0)
        nc.vector.memset(warm[:], 0.0)
        nc.scalar.activation(warm[:], warm[:], AF.Exp, scale=1.0)

        # ones columns of V_aug (whole-tile memset; V parts overwritten later)
        nc.vector.memset(va[:], 1.0)

        if wb_qk:
            nc.sync.dma_start(out=bq_s[:], in_=bqd.ap())
            nc.sync.dma_start(out=bk_s[:], in_=bkd.ap())
        if wb_v:
            nc.sync.dma_start(out=bv_s[:], in_=bvd.ap())
        if wb_o:
            nc.sync.dma_start(out=bo_s[:], in_=bod.ap())

        # input DMAs, arrival order tuned to the schedule
        def load_x(qcg):
            nc.sync.dma_start(
                out=xts[:, qcg],
                in_=xq.ap()[qcg].rearrange("p (k t) -> p k t", k=KT))

        load_x(0)
        nc.sync.dma_start(out=wq_s[:], in_=wq.ap())
        nc.sync.dma_start(out=wk_s[:], in_=wk.ap())
        nc.sync.dma_start(out=wv_s[:], in_=wv.ap())
        for qcg in (1, 2, 3, 4):
            load_x(qcg)
        nc.sync.dma_start(out=wo_s[:], in_=wo.ap())
        for qcg in (5, 6, 7):
            load_x(qcg)

        # A2A buffers per (b, half)
        a2a_in = [dram.tile([NCORES, P, TOK], DT, name=f"a2ai{i}")
                  for i in range(B * HALF)]
        a2a_out = [dram.tile([NCORES, P, TOK], DT, name=f"a2ao{i}")
                   for i in range(B * HALF)]
        # warm-up collective: absorbs first-call staging + NEFF barrier
        wcc_in = dram.tile([NCORES, 16, 16], DT)
        wcc_out = dram.tile([NCORES, 16, 16], DT)
        wcc_sb = sb.tile([16, NCORES * 16], DT)
        nc.vector.memset(wcc_sb[:], 0.0)
        nc.sync.dma_start(out=wcc_in[:].rearrange("j p c -> p j c"),
                          in_=wcc_sb[:].rearrange("p (j c) -> p j c", j=NCORES))
        nc.gpsimd.collective_compute(
            "AllToAll", mybir.AluOpType.bypass,
            replica_groups=[list(range(NCORES))],
            ins=[wcc_in.opt()], outs=[wcc_out.opt()],
        )

        # ---------------- filler generators ----------------
        def gen_qk(qcg, w_s, b_s, dst):
            ps = psum.tile([P, 512], f32, tag="proj", bufs=2, name=f"pp{qcg}")
            if wb_qk:
                nc.tensor.matmul(ps[:], b_s[:], ones_r[:], start=True, stop=False)
            for kti in range(KT):
                nc.tensor.matmul(ps[:], w_s[:, kti], xts[:, qcg, kti],
                                 start=(kti == 0 and not wb_qk),
                                 stop=(kti == KT - 1))
                if kti == 3:
                    yield
            if SCORES_FP8:
                for h in range(2):
                    for jj in range(2):
                        src = ps[h * HD + jj * 32:h * HD + jj * 32 + 32, :]
                        nc.vector.tensor_scalar_max(
                            dst[h * 32:(h + 1) * 32, jj, ts(qcg, 512)],
                            src, 0.0)
            elif PACK_SCORES:
                nc.vector.tensor_scalar_max(dst[:, ts(qcg, 512)], ps[:], 0.0)
            else:
                dsth = qth if dst is qt else kth
                for h in range(2):
                    nc.vector.tensor_scalar_max(dsth[h][:, ts(qcg, 512)],
                                                ps[h * HD:(h + 1) * HD], 0.0)
            yield

        def gen_v(tb):  # tb: global 128-token block 0..31
            vps = psum.tile([P, DC], f32, tag="proj", bufs=2, name=f"pv{tb}")
            if wb_v:
                nc.tensor.matmul(vps[:], ones_r[:, 0:P], bv_s[:],
                                 start=True, stop=False)
            for kti in range(KT):
                nc.tensor.matmul(vps[:], xts[:, tb // 4, kti, ts(tb % 4, P)],
                                 wv_s[:, kti],
                                 start=(kti == 0 and not wb_v),
                                 stop=(kti == KT - 1))
                if kti == 3:
                    yield
            for h in range(2):
                nc.vector.tensor_scalar_max(va[:, tb // 2, h, tb % 2, HD:P],
                                            vps[:, h * HD:(h + 1) * HD], 0.0)
            yield

        def gen_outproj(b, half):
            bh = b * HALF + half
            for ec in range(D // 512):
                ps = psum.tile([P, 512], f32, tag="proj", bufs=2,
                               name=f"po{bh}_{ec}")
                if wb_o:
                    nc.tensor.matmul(ps[:], ones_r[:, 0:P], bo_s[:, ts(ec, 512)],
                                     start=True, stop=False)
                for kti in range(KT):
                    nc.tensor.matmul(ps[:], ctxt[:, bh, kti],
                                     wo_s[:, kti, ts(ec, 512)],
                                     start=(kti == 0 and not wb_o),
                                     stop=(kti == KT - 1))
                    if kti == 3:
                        yield
                osb = osb_p.tile([P, 512], f32, tag="osb")
                nc.vector.tensor_scalar_max(osb[:], ps[:], 0.0)
                nc.sync.dma_start(out=out.ap()[ds(bh * P, P), ts(ec, 512)],
                                  in_=osb[:])
                yield

        def gen_dummy(n):
            # small units (1 MM each) so a lagging PE is never blocked long
            for i in range(n):
                dps = psum.tile([P, 512], f32, tag="proj", bufs=2,
                                name=f"dwm{i}")
                nc.tensor.matmul(dps[:], wo_s[:, i % KT, 0:P],
                                 xts[:, i % (SBQ * B), i % KT],
                                 start=True, stop=True)
                yield

        _dummy_ctr = [0]

        def pad_mm():
            i = _dummy_ctr[0]
            _dummy_ctr[0] += 1
            dps = psum.tile([P, 512], f32, tag="proj", bufs=2,
                            name=f"pad{i}")
            nc.tensor.matmul(dps[:], wo_s[:, i % KT, 0:P],
                             xts[:, i % (SBQ * B), i % KT],
                             start=True, stop=True)

        class Fillers:
            def __init__(self, gens, pad=False):
                self.gens = list(gens)
                self.i = 0
                self.pad = pad

            def emit(self, n):
                got = 0
                while n > got and self.i < len(self.gens):
                    try:
                        next(self.gens[self.i])
                        got += 1
                    except StopIteration:
                        self.i += 1
                if self.pad:
                    # keep the PE fed so HAM doesn't re-throttle the clock
                    for _ in range(n - got):
                        pad_mm()

            def drain(self):
                while self.i < len(self.gens):
                    try:
                        next(self.gens[self.i])
                    except StopIteration:
                        self.i += 1

        # ---------------- attention ----------------
        def attention(b, fillers, sync_gather_last=False, light=False):
            for qc in range(SBQ):
                qsl = ds(b * S + qc * 512, 512)
                cps = [psum.tile([P, 512], f32, tag="ctx", bufs=2,
                                 name=f"c{b}_{qc}_{h}") for h in range(2)]
                for kbp in range(KBP):
                    # pt: [p, j(kb of pair), h, q]; per-head slice [:, :, h]
                    # is the DoubleRow [K, 2, N] layout
                    pt = ptp.tile([P, 2, 2, 512], PDT, tag="p", name="pt")
                    for j in range(2):
                        kb = kbp * 2 + j
                        ksl = ds(b * S + kb * P, P)
                        # heads in different PSUM banks of one tile: the two
                        # score MMs (row groups 0/64) issue adjacently and
                        # can overlap on the PE array
                        sps = psum.tile([P, 2, 512], f32, tag="sc", bufs=2,
                                        name=f"s{b}_{qc}_{kb}")
                        for h in range(2):
                            hs = slice(h * HD, (h + 1) * HD)
                            nc.tensor.matmul(sps[:, h], kt[hs, ksl],
                                             qt[hs, qsl],
                                             start=True, stop=True)
                        # fp8e4 max-normal is 240 on TRN: shift logits by -2
                        # (softmax shift-invariance cancels the factor)
                        if CTX_FP8:
                            nc.scalar.activation(pt[:, j], sps[:], AF.Exp,
                                                 scale=0.125, bias=ebias[:])
                        else:
                            nc.scalar.activation(pt[:, j], sps[:], AF.Exp,
                                                 scale=0.125)
                    for h in range(2):
                        if CTX_FP8:
                            nc.tensor.matmul(cps[h][:], va[:, b * KBP + kbp, h],
                                             pt[:, :, h], start=(kbp == 0),
                                             stop=(kbp == KBP - 1),
                                             perf_mode=DR)
                        else:
                            for j in range(2):
                                kb = kbp * 2 + j
                                nc.tensor.matmul(
                                    cps[h][:],
                                    va[:, b * KBP + kbp, h, j],
                                    pt[:, j, h],
                                    start=(kb == 0), stop=(kb == KB - 1))
                    fillers.emit(1 if (light and kbp % 2) else 2)
                # normalize straight out of PSUM; scatter slivers.
                # denominator is at rows 0:64 (base 0) by va layout, so the
                # fast reciprocal reads it directly; ctx is at rows 64:128
                csb = nrm.tile([P, 512], DT, tag="csb", name=f"csb{b}_{qc}")
                for h in range(2):
                    rec = nrm.tile([HD, 512], f32, tag="rec", name=f"rec{h}")
                    if FAST_RECIP:
                        nc.vector.reciprocal_approx_fast(rec[:],
                                                         cps[h][0:HD, :])
                    else:
                        nc.vector.reciprocal(rec[:], cps[h][0:HD, :])
                    nc.vector.tensor_tensor(csb[h * HD:(h + 1) * HD, :],
                                            cps[h][HD:P, :], rec[:],
                                            mybir.AluOpType.mult)
                    # ship this head's sliver rows while the other head
                    # still normalizes
                    half = qc // 2
                    bh = b * HALF + half
                    q4 = (qc % 2) * 4
                    hs = slice(h * HD, (h + 1) * HD)
                    nc.sync.dma_start(
                        out=a2a_in[bh][q4:q4 + 4, hs].rearrange(
                            "j p c -> p j c"),
                        in_=csb[hs, :].rearrange("p (j c) -> p j c", j=4))
                if qc % 2 == 1:
                    nc.gpsimd.collective_compute(
                        "AllToAll", mybir.AluOpType.bypass,
                        replica_groups=[list(range(NCORES))],
                        ins=[a2a_in[bh].opt()], outs=[a2a_out[bh].opt()],
                    )
                    ge = nc.sync if (sync_gather_last and qc == SBQ - 1) \
                        else nc.gpsimd
                    for i in range(NCORES):
                        ge.dma_start(out=ctxt[:, bh, i, :],
                                     in_=a2a_out[bh][i])

        # ---------------- schedule ----------------
        # lead-in: batch-0 k, v, q(qc0)
        lead = Fillers([gen_qk(0, wq_s, bq_s, qt),
                        gen_qk(0, wk_s, bk_s, kt),
                        gen_v(0), gen_v(1), gen_v(2), gen_v(3),
                        gen_qk(1, wk_s, bk_s, kt),
                        gen_v(4), gen_v(5), gen_v(6), gen_v(7),
                        gen_qk(2, wk_s, bk_s, kt),
                        gen_v(8), gen_v(9), gen_v(10), gen_v(11),
                        gen_qk(3, wk_s, bk_s, kt),
                        gen_v(12), gen_v(13), gen_v(14), gen_v(15)])
        lead.drain()

        # one shared filler queue across both attention phases; order =
        # priority (deps still enforced by the scheduler)
        fq = Fillers(pad=PAD_PE, gens=[
            gen_qk(1, wq_s, bq_s, qt),
            gen_qk(4, wk_s, bk_s, kt), gen_v(16), gen_v(17),
            gen_qk(2, wq_s, bq_s, qt),
            gen_qk(5, wk_s, bk_s, kt), gen_v(18), gen_v(19),
            gen_qk(3, wq_s, bq_s, qt),
            gen_qk(4, wq_s, bq_s, qt),
            gen_qk(6, wk_s, bk_s, kt), gen_v(20), gen_v(21),
            gen_qk(7, wk_s, bk_s, kt), gen_v(22), gen_v(23),
            gen_qk(5, wq_s, bq_s, qt), gen_v(24), gen_v(25),
            gen_v(26), gen_v(27),
            gen_qk(6, wq_s, bq_s, qt), gen_v(28), gen_v(29),
            gen_v(30), gen_v(31),
            gen_qk(7, wq_s, bq_s, qt),
        ])
        attention(0, fq)
        # leftovers spill into attention(1); keep attention(1) light so the
        # exp pace is not PE-gated
        attention(1, fq, sync_gather_last=True)
        fq.drain()

        # tail: all output projections; batch-0's ride the A2A(1,1) wait
        g_t = Fillers([gen_outproj(0, 0), gen_outproj(0, 1),
                       gen_outproj(1, 0), gen_outproj(1, 1)])
        g_t.drain()

        if DEBUG_DUMP:
            if DEBUG_DUMP == 2:
                nc.sync.dma_start(
                    out=dbg.ap(),
                    in_=va[:].rearrange("p a h j m -> p (a h j m)"))
            else:
                nc.sync.dma_start(
                    out=dbg.ap()[:, 0:B * HALF * KT * TOK],
                    in_=ctxt[:].rearrange("p a k t -> p (a k t)"))

    nc.compile()
    return nc


def _get(wb_qk, wb_v, wb_o):
    key = (wb_qk, wb_v, wb_o)
    if key not in _CACHE:
        _CACHE[key] = _build(*key)
    return _CACHE[key]


def kernel(x, Wq, bq, Wk, bk, Wv, bv, Wo, bo):
    global LAST_RESULTS
    from concourse.bass_utils import run_bass_kernel_spmd

    x = np.asarray(x, dtype=np.float32)
    Wq, Wk, Wv, Wo = (np.asarray(w, dtype=np.float32) for w in (Wq, Wk, Wv, Wo))
    bq, bk, bv, bo = (np.asarray(v, dtype=np.float32) for v in (bq, bk, bv, bo))

    wb_qk = bool(np.any(bq) or np.any(bk))
    wb_v = bool(np.any(bv))
    wb_o = bool(np.any(bo))
    nc = _get(wb_qk, wb_v, wb_o)

    # pack x: [T, D] -> xT [D, T] -> [qcg, kti, p, 512] -> [qcg, p, kti*512]
    xT = x.reshape(T, D).astype(_bf).T                      # [D, T]
    xq_full = np.ascontiguousarray(
        xT.reshape(KT, P, SBQ * B, 512).transpose(2, 1, 0, 3)
          .reshape(SBQ * B, P, KT * 512))
    Wo16 = np.ascontiguousarray(
        Wo.astype(_bf).reshape(KT, P, D).transpose(1, 0, 2))  # [P, KT, D]
    bo16 = np.ascontiguousarray(bo.astype(_bf).reshape(1, D))

    def pack_w(W, c):
        cs = slice(c * DC, (c + 1) * DC)
        return np.ascontiguousarray(
            W[:, cs].astype(_bf).reshape(KT, P, DC).transpose(1, 0, 2))

    in_maps = []
    for c in range(NCORES):
        cs = slice(c * DC, (c + 1) * DC)
        in_maps.append({
            "xq": xq_full,
            "wq": pack_w(Wq, c),
            "wk": pack_w(Wk, c),
            "wv": pack_w(Wv, c),
            "wo": Wo16,
            "bqv": np.ascontiguousarray(bq[cs].astype(_bf).reshape(1, DC)),
            "bkv": np.ascontiguousarray(bk[cs].astype(_bf).reshape(1, DC)),
            "bvv": np.ascontiguousarray(bv[cs].astype(_bf).reshape(1, DC)),
            "bov": bo16,
        })

    kw = {}
    if PROFILE:
        kw = dict(trace=True, trace_cores=PROFILE_CORES)
    res = run_bass_kernel_spmd(nc, in_maps, core_ids=list(range(NCORES)), **kw)
    LAST_RESULTS = res

    # core j's out rows: (b*2+half)*128 + i -> token b*S + half*1024 + j*128 + i
    full = np.empty((B, S, D), np.float32)
    for j in range(NCORES):
        o = res.results[j]["out"]
        for b in range(B):
            for half in range(HALF):
                r0 = (b * HALF + half) * TOK
                t0 = half * 1024 + j * TOK
                full[b, t0:t0 + TOK] = o[r0:r0 + TOK]
    return np.ascontiguousarray(full)
